# revision 3
# baseline (speedup 1.0000x reference)
"""Row-normalize block-diagonal graph weights on 8 Trainium2 NeuronCores.

fp16 I/O (rel-err budget 2e-2 dwarfs fp16 rounding ~5e-4): host downcasts
edge_weight, device streams 8MB in + 8MB out per core, host upcasts.

Per-core pipeline over 10 chunks (2,2,4,4,4,4,4,4,2,2 graph-row columns):
  SP ring:  all 10 loads dispatched up-front, back to back (single HWDGE
            queue; splitting loads across queues lowers union bandwidth)
  DVE:      per-chunk row sums via in-place tensor_scalar mul-by-1.0 with
            accum_out (runs in the 2x/4x 16-bit perf mode, vs 1x for
            TENSOR_REDUCE), then 1/x and the normalize multiplies
  ACT:      store dispatch only, one per chunk as DVE signals, on the
            second HWDGE ring (so loads and stores stream concurrently)
Sync is 4 cumulative semaphores (s_in/s_st/s_dn + spare); HWDGE completes
in FIFO order per ring so chunk i is awaited as sem >= 16*(i+1).

Device applies no zero-degree clamp and assumes row == arange//N; the host
routes every element of any row where that fails (foreign contributions,
misrouted elements, or near-zero true degree) through an exact fixup path.

Sharding: pure data parallel over K — each core owns 4 graphs
([4096, 1024] slab); no cross-core communication.
"""

import numpy as np

K = 32          # graphs in batch
N = 1024        # nodes per graph
NCORES = 8
KPC = K // NCORES          # graphs per core
ROWS = KPC * N             # 4096 source-node rows per core
NODES = K * N              # total segments
P = 128                    # SBUF partitions
Q = 4                      # consecutive rows per partition per slab
T = ROWS // (Q * P)        # 8 slabs per core

_CACHE = {}


def _build_bass():
    if "nc" in _CACHE:
        return _CACHE["nc"]

    import concourse.bass as bass
    from concourse import mybir

    f32 = mybir.dt.float32
    f16 = mybir.dt.float16
    nc = bass.Bass("TRN2", target_bir_lowering=False, debug=False,
                   num_devices=NCORES)
    x = nc.dram_tensor("x", [ROWS, N], f16, kind="ExternalInput").ap()
    y = nc.dram_tensor("y", [ROWS, N], f16, kind="ExternalOutput").ap()
    # slab t covers rows [t*P*Q, (t+1)*P*Q): partition p holds Q
    # consecutive DRAM rows -> one contiguous (Q*N*2)B run per partition
    xt = x.rearrange("(t p q) n -> t p (q n)", p=P, q=Q)
    yt = y.rearrange("(t p q) n -> t p (q n)", p=P, q=Q)

    with (
        nc.sbuf_tensor([P, T * Q * N], f16) as wall,
        nc.sbuf_tensor([P, T * Q], f32) as degall,
        nc.sbuf_tensor([P, T * Q], f32) as invall,
        nc.semaphore("s_in") as s_in,
        nc.semaphore("s_st") as s_st,
        nc.semaphore("s_dn") as s_dn,
        nc.Block(no_gpsimd_drain=True) as block,
    ):
        wap = wall.ap()
        degap, invap = degall.ap(), invall.ap()

        # (slab, q0, qc): small chunks at both ends of the pipeline
        chunks = ([(0, 0, 2), (0, 2, 2)]
                  + [(t, 0, 4) for t in range(1, 7)]
                  + [(7, 0, 2), (7, 2, 2)])

        def wslice(t, q0, qc):
            base = t * Q * N + q0 * N
            return wap[:, base:base + qc * N]

        @block.sync
        def _(sync):
            for t, q0, qc in chunks:
                sync.dma_start(out=wslice(t, q0, qc),
                               in_=xt[t][:, q0 * N:(q0 + qc) * N]
                               ).then_inc(s_in, 16)

        import os
        sum_mode = os.environ.get("KSUM", "ts_accum")

        @block.vector
        def _(vector):
            for i, (t, q0, qc) in enumerate(chunks):
                vector.wait_ge(s_in, 16 * (i + 1))
                for q in range(q0, q0 + qc):
                    col = t * Q + q
                    if sum_mode == "reduce":
                        vector.reduce_sum(out=degap[:, col:col + 1],
                                          in_=wap[:, col * N:(col + 1) * N],
                                          axis=mybir.AxisListType.X)
                        continue
                    # in-place w *= 1.0 whose accumulator emits the row
                    # sum: single-src 16-bit tensor_scalar runs in the
                    # fast DVE perf mode, unlike TENSOR_REDUCE (1x only)
                    vector.tensor_scalar(
                        out=wap[:, col * N:(col + 1) * N],
                        in0=wap[:, col * N:(col + 1) * N],
                        scalar1=1.0, scalar2=None,
                        op0=mybir.AluOpType.mult,
                        accum_out=degap[:, col:col + 1])
                # DVE is a deep pipeline without interlocks: drain
                # between same-engine RAW-dependent ops
                base = t * Q + q0
                vector.drain()
                vector.reciprocal(out=invap[:, base:base + qc],
                                  in_=degap[:, base:base + qc])
                vector.drain()
                for q in range(q0, q0 + qc):
                    col = t * Q + q
                    vector.tensor_scalar_mul(
                        wap[:, col * N:(col + 1) * N],
                        wap[:, col * N:(col + 1) * N],
                        invap[:, col:col + 1])
                # drain before signalling: the muls' SBUF writes must
                # be visible to the SDMA engines before the store
                vector.drain().then_inc(s_st, 1)

        @block.scalar
        def _(scalar):
            for i, (t, q0, qc) in enumerate(chunks):
                scalar.wait_ge(s_st, i + 1)
                scalar.dma_start(out=yt[t][:, q0 * N:(q0 + qc) * N],
                                 in_=wslice(t, q0, qc)).then_inc(s_dn, 16)
            scalar.wait_ge(s_dn, 16 * len(chunks))

    _CACHE["nc"] = nc
    return nc


def _expected_row_pattern():
    if "base" not in _CACHE:
        _CACHE["base"] = (np.arange(K * N * N, dtype=np.int64) // N)
    return _CACHE["base"]


def _install_ntff_hook():
    """Recreate the NTFF profile hook the boot shim couldn't install
    (this image's antenv lacks axon_hooks). Safe no-op on failure."""
    import sys, types
    if "antenv.axon_hooks" in sys.modules:
        return
    try:
        from trn_agent_boot.trn_boot import _ntff_profile_via_ctypes
        hook = _ntff_profile_via_ctypes("/opt/axon/libaxon_pjrt.so")
        mod = types.ModuleType("antenv.axon_hooks")
        mod.get_axon_ntff_profile_hook = lambda: hook
        mod.set_axon_ntff_profile_hook = lambda h: None
        sys.modules["antenv.axon_hooks"] = mod
    except Exception:
        pass


def _run_spmd(edge_weight, trace=False):
    from concourse.bass_utils import run_bass_kernel_spmd

    if trace:
        _install_ntff_hook()
    nc = _build_bass()
    ew = np.asarray(edge_weight)
    ew16 = np.ascontiguousarray(ew.astype(np.float16))
    in_maps = [{"x": ew16[c * KPC:(c + 1) * KPC].reshape(ROWS, N)}
               for c in range(NCORES)]
    res = run_bass_kernel_spmd(nc, in_maps, list(range(NCORES)), trace=trace)
    out = np.empty((K, N * N), dtype=np.float32)
    for c in range(NCORES):
        out[c * KPC:(c + 1) * KPC] = \
            res.results[c]["y"].astype(np.float32).reshape(KPC, N * N)
    return out, res


def _prepare(edge_weight, row):
    """Host-side exact handling of every element the device cannot
    produce. The device assumes row[e] == e//N and applies no
    zero-degree clamp, so the fixup set is: all misrouted elements,
    plus every element of any row whose true degree differs from the
    device's blockwise sum (foreign contributions / lost elements) or
    is ~0 (the unclamped device 1/x would blow up).

    Returns (fixup_idx int64, fixup_val f32): out[fixup_idx] = fixup_val
    reproduces deg_inv[clamped row] * w exactly for those elements.
    """
    w = edge_weight.reshape(-1)
    base = _expected_row_pattern()
    row = row.astype(np.int64, copy=False)
    mis = np.flatnonzero(row != base)
    dev_deg = edge_weight.reshape(NODES, N).sum(axis=1, dtype=np.float64)
    true_deg = dev_deg.copy()
    if mis.size:
        wE = w[mis].astype(np.float64)
        np.subtract.at(true_deg, base[mis], wE)
        rE = row[mis]
        valid = (rE >= 0) & (rE < NODES)
        np.add.at(true_deg, rE[valid], wE[valid])
    true32 = true_deg.astype(np.float32)
    inv = np.where(true32 > 0, np.float32(1.0) / true32, np.float32(0.0))
    bad = np.flatnonzero((true_deg != dev_deg) | (true32 < np.float32(1e-3)))
    E = mis
    if bad.size:
        elems = (bad[:, None] * N + np.arange(N)[None, :]).reshape(-1)
        E = np.unique(np.concatenate([E, elems]))
    if E.size:
        gather = np.clip(row[E], 0, NODES - 1)   # jnp OOB gather clamps
        fixup_val = (w[E] * inv[gather]).astype(np.float32)
    else:
        fixup_val = np.zeros(0, dtype=np.float32)
    return E, fixup_val


def kernel(edge_weight, row, num_atom):
    edge_weight = np.asarray(edge_weight)
    row = np.asarray(row)
    if (edge_weight.shape != (K, N * N)
            or int(num_atom) != N
            or row.shape != (K * N * N,)):
        return _numpy_reference(edge_weight, row, int(num_atom))
    E, fixup_val = _prepare(edge_weight, row)
    out, _ = _run_spmd(edge_weight)
    if E.size:
        out.reshape(-1)[E] = fixup_val
    return out


def _numpy_reference(edge_weight, row, num_atom):
    """jnp-semantics fallback for unexpected shapes: scatter drops OOB,
    gather clamps."""
    Kb = edge_weight.shape[0]
    num_nodes = Kb * num_atom
    w = edge_weight.reshape(-1).astype(np.float32)
    row = row.astype(np.int64, copy=False)
    valid = (row >= 0) & (row < num_nodes)
    deg = np.zeros(num_nodes, dtype=np.float64)
    np.add.at(deg, row[valid], w[valid].astype(np.float64))
    deg = deg.astype(np.float32)
    deg_inv = np.where(deg > 0, np.float32(1.0) / deg, np.float32(0.0))
    out = deg_inv[np.clip(row, 0, num_nodes - 1)] * w
    return out.reshape(Kb, -1).astype(np.float32)


def bench(edge_weight, row, num_atom, trace=True):
    """Like kernel() but returns (output, BassKernelResults) with profiling."""
    edge_weight = np.asarray(edge_weight)
    row = np.asarray(row)
    E, fixup_val = _prepare(edge_weight, row)
    out, res = _run_spmd(edge_weight, trace=trace)
    if E.size:
        out.reshape(-1)[E] = fixup_val
    return out, res


# revision 9
# speedup vs baseline: 1.3577x; 1.3577x over previous
"""Row-normalize block-diagonal graph weights on 8 Trainium2 NeuronCores.

fp16 I/O (rel-err budget 2e-2 dwarfs fp16 rounding ~5e-4): host downcasts
edge_weight, device streams 8MB in + 8MB out per core, host upcasts.

Per-core pipeline over 10 chunks (2,2,4,4,4,4,4,4,2,2 graph-row columns):
  SP ring:  all 10 loads dispatched up-front, back to back (single HWDGE
            queue; splitting loads across queues lowers union bandwidth)
  ACT:      row sums for the first half of each chunk's columns
            (ACTIVATE Copy + accum_out, ~1.4us/col) and every store
            dispatch (two chunks behind) on the second HWDGE ring
  DVE:      row sums for the other columns via double-fold 16-bit TT
            adds (2x mode) + short TENSOR_REDUCE (~0.8us/col vs 1.2us
            for a bare 1x TENSOR_REDUCE), then 1/x and all the
            normalize multiplies (tensor_scalar, ~0.5us/col)
Sync is 4 cumulative semaphores; HWDGE completes in FIFO order per ring
so chunk i's load is awaited as s_in >= 16*(i+1). Same-engine RAW needs
no explicit drain (the DVE pipe flushes between ops); drains appear only
before cross-engine semaphore increments.

Device applies no zero-degree clamp and assumes row == arange//N; the host
routes every element of any row where that fails (foreign contributions,
misrouted elements, or near-zero true degree) through an exact fixup path.

Sharding: pure data parallel over K — each core owns 4 graphs
([4096, 1024] slab); no cross-core communication.
"""

import numpy as np

K = 32          # graphs in batch
N = 1024        # nodes per graph
NCORES = 8
KPC = K // NCORES          # graphs per core
ROWS = KPC * N             # 4096 source-node rows per core
NODES = K * N              # total segments
P = 128                    # SBUF partitions
Q = 4                      # consecutive rows per partition per slab
T = ROWS // (Q * P)        # 8 slabs per core

_CACHE = {}


def _build_bass():
    if "nc" in _CACHE:
        return _CACHE["nc"]

    import concourse.bass as bass
    from concourse import mybir

    f32 = mybir.dt.float32
    f16 = mybir.dt.float16
    nc = bass.Bass("TRN2", target_bir_lowering=False, debug=False,
                   num_devices=NCORES)
    x = nc.dram_tensor("x", [ROWS, N], f16, kind="ExternalInput").ap()
    y = nc.dram_tensor("y", [ROWS, N], f16, kind="ExternalOutput").ap()
    # slab t covers rows [t*P*Q, (t+1)*P*Q): partition p holds Q
    # consecutive DRAM rows -> one contiguous (Q*N*2)B run per partition
    xt = x.rearrange("(t p q) n -> t p (q n)", p=P, q=Q)
    yt = y.rearrange("(t p q) n -> t p (q n)", p=P, q=Q)

    with (
        nc.sbuf_tensor([P, T * Q * N], f16) as wall,
        nc.sbuf_tensor([P, 2 * 512], f16) as fold1,
        nc.sbuf_tensor([P, 2 * 256], f16) as fold2,
        nc.sbuf_tensor([P, T * Q], f32) as degall,
        nc.sbuf_tensor([P, T * Q], f32) as invall,
        nc.semaphore("s_in") as s_in,
        nc.semaphore("s_deg") as s_deg,
        nc.semaphore("s_st") as s_st,
        nc.semaphore("s_dn") as s_dn,
        nc.Block(no_gpsimd_drain=True) as block,
    ):
        wap = wall.ap()
        degap, invap = degall.ap(), invall.ap()
        f1 = fold1.ap().rearrange("p (c m) -> p c m", c=2)
        f2 = fold2.ap().rearrange("p (c m) -> p c m", c=2)

        # (slab, q0, qc): small chunks at both ends of the pipeline
        chunks = ([(0, 0, 2), (0, 2, 2)]
                  + [(t, 0, 4) for t in range(1, 7)]
                  + [(7, 0, 2), (7, 2, 2)])
        NCH = len(chunks)
        # per-chunk split: ACT sums the first half of the columns
        # (ACTIVATE+accum, ~1.4us/col), DVE the rest (double-fold TT
        # adds + short reduce, ~0.8us/col vs 1.2us for a bare
        # TENSOR_REDUCE) plus the reciprocal and all the multiplies
        nact = {2: 1, 4: 2}

        def wcol(col):
            return wap[:, col * N:(col + 1) * N]

        def wslice(t, q0, qc):
            base = t * Q * N + q0 * N
            return wap[:, base:base + qc * N]

        @block.sync
        def _(sync):
            for t, q0, qc in chunks:
                sync.dma_start(out=wslice(t, q0, qc),
                               in_=xt[t][:, q0 * N:(q0 + qc) * N]
                               ).then_inc(s_in, 16)

        @block.vector
        def _(vector):
            # Software-pipelined: iteration i sums chunk i, takes 1/x of
            # chunk i-1 and multiplies chunk i-2. The DVE pipe has no
            # RAW interlock for short distances (a consumer's first read
            # can beat the producer's last SBUF write by ~200 cycles),
            # so every dependent pair is separated by a full unrelated
            # op stream; independent muls double as separators inside an
            # iteration, with drains as fillers in the first iterations.
            def mul(col):
                vector.tensor_scalar_mul(wcol(col), wcol(col),
                                         invap[:, col:col + 1])

            for i in range(NCH + 2):
                seps = []        # separator work: muls of chunk i-2
                if i >= 2:
                    tm, qm0, qmc = chunks[i - 2]
                    seps = [tm * Q + q for q in range(qm0, qm0 + qmc)]

                def sep(k):
                    if k < len(seps):
                        mul(seps[k])
                    else:
                        vector.drain()

                if i < NCH:
                    t, q0, qc = chunks[i]
                    vector.wait_ge(s_in, 16 * (i + 1))
                    nd = qc - nact[qc]           # cols summed on DVE
                    c0 = t * Q + q0 + nact[qc]   # first DVE col
                    wv = wslice(t, q0 + nact[qc], nd) \
                        .rearrange("p (c m) -> p c m", c=nd)
                    vector.tensor_add(f1[:, :nd], wv[:, :, 0:512],
                                      wv[:, :, 512:1024])
                    sep(0)
                    vector.tensor_add(f2[:, :nd], f1[:, :nd, 0:256],
                                      f1[:, :nd, 256:512])
                    sep(1)
                    vector.reduce_sum(out=degap[:, c0:c0 + nd],
                                      in_=f2[:, :nd],
                                      axis=mybir.AxisListType.X)
                else:
                    sep(0)
                    sep(1)
                if 0 <= i - 1 < NCH:
                    tr, qr0, qrc = chunks[i - 1]
                    vector.wait_ge(s_deg, i)
                    br = tr * Q + qr0
                    vector.reciprocal(out=invap[:, br:br + qrc],
                                      in_=degap[:, br:br + qrc])
                for k in range(2, len(seps)):
                    mul(seps[k])
                if i >= 2:
                    # drain before signalling: the muls' SBUF writes
                    # must be visible to the SDMA store engines
                    vector.drain().then_inc(s_st, 1)

        @block.scalar
        def _(scalar):
            # dummy activate hoists the lazy ACT_TABLE_LOAD (~1.3us)
            # into the load-DMA window instead of the first real sum
            scalar.activation(invap[:, 0:1], degap[:, 0:1],
                              mybir.ActivationFunctionType.Copy)

            def disp(j):
                tj, qj0, qjc = chunks[j]
                scalar.wait_ge(s_st, j + 1)
                scalar.dma_start(out=yt[tj][:, qj0 * N:(qj0 + qjc) * N],
                                 in_=wslice(tj, qj0, qjc)
                                 ).then_inc(s_dn, 16)

            for i, (t, q0, qc) in enumerate(chunks):
                scalar.wait_ge(s_in, 16 * (i + 1))
                for q in range(q0, q0 + nact[qc]):
                    col = t * Q + q
                    scalar.activation(
                        wcol(col), wcol(col),
                        mybir.ActivationFunctionType.Copy,
                        accum_out=degap[:, col:col + 1])
                scalar.drain().then_inc(s_deg, 1)
                # stagger 3 matches the DVE software pipeline (muls for
                # chunk j land in DVE iteration j+2) so ACT never blocks
                if i >= 3:
                    disp(i - 3)
            for j in range(NCH - 3, NCH):
                disp(j)
            scalar.wait_ge(s_dn, 16 * NCH)

    _CACHE["nc"] = nc
    return nc


def _expected_row_pattern():
    if "base" not in _CACHE:
        _CACHE["base"] = (np.arange(K * N * N, dtype=np.int64) // N)
    return _CACHE["base"]


def _install_ntff_hook():
    """Recreate the NTFF profile hook the boot shim couldn't install
    (this image's antenv lacks axon_hooks). Safe no-op on failure."""
    import sys, types
    if "antenv.axon_hooks" in sys.modules:
        return
    try:
        from trn_agent_boot.trn_boot import _ntff_profile_via_ctypes
        hook = _ntff_profile_via_ctypes("/opt/axon/libaxon_pjrt.so")
        mod = types.ModuleType("antenv.axon_hooks")
        mod.get_axon_ntff_profile_hook = lambda: hook
        mod.set_axon_ntff_profile_hook = lambda h: None
        sys.modules["antenv.axon_hooks"] = mod
    except Exception:
        pass


def _run_spmd(edge_weight, trace=False):
    from concourse.bass_utils import run_bass_kernel_spmd

    if trace:
        _install_ntff_hook()
    nc = _build_bass()
    ew = np.asarray(edge_weight)
    ew16 = np.ascontiguousarray(ew.astype(np.float16))
    in_maps = [{"x": ew16[c * KPC:(c + 1) * KPC].reshape(ROWS, N)}
               for c in range(NCORES)]
    res = run_bass_kernel_spmd(nc, in_maps, list(range(NCORES)), trace=trace)
    out = np.empty((K, N * N), dtype=np.float32)
    for c in range(NCORES):
        out[c * KPC:(c + 1) * KPC] = \
            res.results[c]["y"].astype(np.float32).reshape(KPC, N * N)
    return out, res


def _prepare(edge_weight, row):
    """Host-side exact handling of every element the device cannot
    produce. The device assumes row[e] == e//N and applies no
    zero-degree clamp, so the fixup set is: all misrouted elements,
    plus every element of any row whose true degree differs from the
    device's blockwise sum (foreign contributions / lost elements) or
    is ~0 (the unclamped device 1/x would blow up).

    Returns (fixup_idx int64, fixup_val f32): out[fixup_idx] = fixup_val
    reproduces deg_inv[clamped row] * w exactly for those elements.
    """
    w = edge_weight.reshape(-1)
    base = _expected_row_pattern()
    row = row.astype(np.int64, copy=False)
    mis = np.flatnonzero(row != base)
    dev_deg = edge_weight.reshape(NODES, N).sum(axis=1, dtype=np.float64)
    true_deg = dev_deg.copy()
    if mis.size:
        wE = w[mis].astype(np.float64)
        np.subtract.at(true_deg, base[mis], wE)
        rE = row[mis]
        valid = (rE >= 0) & (rE < NODES)
        np.add.at(true_deg, rE[valid], wE[valid])
    true32 = true_deg.astype(np.float32)
    inv = np.where(true32 > 0, np.float32(1.0) / true32, np.float32(0.0))
    bad = np.flatnonzero((true_deg != dev_deg) | (true32 < np.float32(1e-3)))
    E = mis
    if bad.size:
        elems = (bad[:, None] * N + np.arange(N)[None, :]).reshape(-1)
        E = np.unique(np.concatenate([E, elems]))
    if E.size:
        gather = np.clip(row[E], 0, NODES - 1)   # jnp OOB gather clamps
        fixup_val = (w[E] * inv[gather]).astype(np.float32)
    else:
        fixup_val = np.zeros(0, dtype=np.float32)
    return E, fixup_val


def kernel(edge_weight, row, num_atom):
    edge_weight = np.asarray(edge_weight)
    row = np.asarray(row)
    if (edge_weight.shape != (K, N * N)
            or int(num_atom) != N
            or row.shape != (K * N * N,)):
        return _numpy_reference(edge_weight, row, int(num_atom))
    E, fixup_val = _prepare(edge_weight, row)
    out, _ = _run_spmd(edge_weight)
    if E.size:
        out.reshape(-1)[E] = fixup_val
    return out


def _numpy_reference(edge_weight, row, num_atom):
    """jnp-semantics fallback for unexpected shapes: scatter drops OOB,
    gather clamps."""
    Kb = edge_weight.shape[0]
    num_nodes = Kb * num_atom
    w = edge_weight.reshape(-1).astype(np.float32)
    row = row.astype(np.int64, copy=False)
    valid = (row >= 0) & (row < num_nodes)
    deg = np.zeros(num_nodes, dtype=np.float64)
    np.add.at(deg, row[valid], w[valid].astype(np.float64))
    deg = deg.astype(np.float32)
    deg_inv = np.where(deg > 0, np.float32(1.0) / deg, np.float32(0.0))
    out = deg_inv[np.clip(row, 0, num_nodes - 1)] * w
    return out.reshape(Kb, -1).astype(np.float32)


def bench(edge_weight, row, num_atom, trace=True):
    """Like kernel() but returns (output, BassKernelResults) with profiling."""
    edge_weight = np.asarray(edge_weight)
    row = np.asarray(row)
    E, fixup_val = _prepare(edge_weight, row)
    out, res = _run_spmd(edge_weight, trace=trace)
    if E.size:
        out.reshape(-1)[E] = fixup_val
    return out, res
